# revision 67
# baseline (speedup 1.0000x reference)
"""Multi-head attention kernel for Trainium2, SPMD over 8 NeuronCores.

Problem: B=2, S=2048, E=1024, H=16 heads, Dh=64.
  q = per-head q_in @ Wq.T (Wq shared across heads), same for k, v
  attn = softmax(q k^T / 8); ctx = attn @ v; out = concat(ctx) @ Wo.T + bo

Sharding: core c handles batch b=c//4 and heads 4*(c%4)..4*(c%4)+3
(head-parallel attention).  The out projection is sharded by e_out rows
(each core owns 256 rows of Wo), with AllGathers of the per-head context
over the 4 cores of each batch group in between.

All layout work happens host-side in kernel(): per-head transposed bf16
q/k ([Dh, head, S]), V repacked per key-chunk with a ones column riding
the contraction for the softmax row-sum, Wo pre-transposed into PE
stationary layout.  The device then does only the module's math:

  A = Wq^T Wk (fused q/k projection), u = A @ qT per head
  scores^T = kT_chunk^T u  ->  exp (ACT)  ->  w2 += vin_ones^T p (PSUM)
  ctx^T = (Wv @ w2) * (1/rowsum)    (normalization commutes with Wv)
  out = woT^T ctx_all + bo          (8 chunks, 2 AllGather rounds)

Schedule: a flat stream of (head, q-range) passes, each a 16-chunk key
loop, keeps ACT (exp) ~97-100% busy through the body; scores are emitted
one unit ahead (across pass boundaries) so exp never queues behind w2 on
the in-order PE queue, w2 of the first two chunks is deferred so the
single w2 PSUM buffer can turn around at pass boundaries, and a filler
queue trickles u/ctx/out-projection matmuls one per chunk into PE's
slack so its p-state stays at full clock.  The last q-half runs as two
quarter passes whose AllGather/out-projection chains overlap the final
attention; a short PE keep-warm chain bridges the tail's DMA-hop
latency.  PSUM: 2x scores double-buffer (4 banks) + w2 accumulator (2)
+ filler aux (2).
"""

import collections
import contextlib
import sys

sys.path.insert(0, "/opt/trn_rl_repo")

import ml_dtypes
import numpy as np

import concourse.bass as bass  # noqa: F401  (bass types via bacc)
import concourse.tile as tile
from concourse import bacc, mybir
from concourse.bass_utils import run_bass_kernel_spmd

B, S, E, H, Dh = 2, 2048, 1024, 16, 64
N_CORES = 8
HPC = 4              # heads per core
NK = S // 128        # 16 key chunks
EOUT = E // 4        # out-projection rows per core
SH = S // 2          # queries per (head, q-half) pass

F32 = mybir.dt.float32
BF16 = mybir.dt.bfloat16
EXP = mybir.ActivationFunctionType.Exp
ADD = mybir.AluOpType.add

_CACHE = {}
_DEBUG = False


def _declare_io(nc):
    io = {}
    io["qT"] = nc.dram_tensor("qT", [Dh, HPC, S], BF16, kind="ExternalInput").ap()
    io["kT"] = nc.dram_tensor("kT", [Dh, HPC, S], BF16, kind="ExternalInput").ap()
    io["vin"] = nc.dram_tensor(
        "vin", [128, HPC, NK, Dh + 1], BF16, kind="ExternalInput"
    ).ap()
    io["wqk"] = nc.dram_tensor("wqk", [Dh, 2, Dh], F32, kind="ExternalInput").ap()
    io["wvT"] = nc.dram_tensor("wvT", [Dh, Dh], BF16, kind="ExternalInput").ap()
    io["woT"] = nc.dram_tensor("woT", [128, 8, EOUT], BF16, kind="ExternalInput").ap()
    io["bo2"] = nc.dram_tensor("bo2", [128, 2], F32, kind="ExternalInput").ap()
    io["outT"] = nc.dram_tensor("outT", [EOUT, S], F32, kind="ExternalOutput").ap()
    if _DEBUG:
        for nm, shape in (
            ("dbg_qt", [Dh, S]), ("dbg_u", [Dh, S]), ("dbg_p", [128, SH]),
            ("dbg_w2", [Dh, SH]), ("dbg_rs", [Dh, SH]), ("dbg_ctx", [Dh, SH]),
        ):
            io[nm] = nc.dram_tensor(nm, shape, F32, kind="ExternalOutput").ap()
    return io


class _Piece:
    """A PE filler item: a few matmuls into one aux-psum tile plus a
    finishing (evacuation) op; emitted one matmul per attention unit."""

    def __init__(self, alloc, mms, fin):
        self.alloc = alloc
        self.mms = list(mms)
        self.fin = fin
        self.tile = None

    def step(self):
        if self.tile is None:
            self.tile = self.alloc()
        self.mms.pop(0)(self.tile)
        if not self.mms:
            if self.fin is not None:
                self.fin(self.tile)
            return True
        return False


class _Fillers:
    def __init__(self):
        self.q = collections.deque()

    def add(self, piece, front=False):
        (self.q.appendleft if front else self.q.append)(piece)

    def pop_one(self):
        if not self.q:
            return
        if self.q[0].step():
            self.q.popleft()

    def drain(self):
        while self.q:
            self.pop_one()

    def finish_front(self):
        """Run the front piece to completion so its aux slot frees."""
        if self.q and self.q[0].tile is not None:
            while not self.q[0].step():
                pass
            self.q.popleft()


def _body(nc, tc, es, io, it, collective=True):
    def pool(name, bufs, space="SBUF"):
        return es.enter_context(
            tc.tile_pool(name=f"{name}_{it}", bufs=bufs, space=space)
        )

    qTd, kTd, vind = io["qT"], io["kT"], io["vin"]
    wqkd, wvTd, woTd, bo2d, outT = (
        io["wqk"], io["wvT"], io["woT"], io["bo2"], io["outT"],
    )

    persist = pool("persist", 1)
    scp = pool("scp", 2, space="PSUM")    # 2x [128,1024] f32 = 4 banks
    w2p = pool("w2p", 1, space="PSUM")    # [65,1024] f32 = 2 banks
    aux = pool("aux", 2, space="PSUM")    # 2x [128,512] f32 = 2 banks
    ppool = pool("ppool", 7)
    w2sbp = pool("w2sbp", 2)
    ctxp = pool("ctxp", 3)
    rsp = pool("rsp", 1)
    osbp = pool("osbp", 2)
    dram = pool("dram", 1, space="DRAM")

    # ---------------- input loads (host-prepped layouts) ----------------
    wqk_sb = persist.tile([Dh, 2, Dh], F32, tag="wqk")
    nc.sync.dma_start(out=wqk_sb[:], in_=wqkd[:, :, :])
    qT = persist.tile([Dh, HPC, S], BF16, tag="qT")
    nc.sync.dma_start(out=qT[:, 0, :], in_=qTd[:, 0, :])
    kT = persist.tile([Dh, HPC, S], BF16, tag="kT")
    nc.sync.dma_start(out=kT[:, 0, :], in_=kTd[:, 0, :])
    vin = persist.tile([128, HPC, NK, Dh + 1], BF16, tag="vin")
    nc.sync.dma_start(out=vin[:, 0, :, :], in_=vind[:, 0, :, :])
    wvT_sb = persist.tile([Dh, Dh], BF16, tag="wvT")
    nc.sync.dma_start(out=wvT_sb[:], in_=wvTd[:, :])
    nc.sync.dma_start(out=qT[:, 1:HPC, :], in_=qTd[:, 1:HPC, :])
    nc.sync.dma_start(out=kT[:, 1:HPC, :], in_=kTd[:, 1:HPC, :])
    nc.sync.dma_start(out=vin[:, 1:HPC, :, :], in_=vind[:, 1:HPC, :, :])
    woT = persist.tile([128, 8, EOUT], BF16, tag="woT")
    nc.sync.dma_start(out=woT[:], in_=woTd[:, :, :])
    bo_sb = persist.tile([128, 2], F32, tag="bo")
    nc.sync.dma_start(out=bo_sb[:], in_=bo2d[:, :])

    # ---------------- PE ramp warm-up ----------------
    # Two tiny matmuls on a zeroed tile start the tensor engine's p-state
    # ramp immediately so the first real matmuls run at speed.
    warm0 = persist.tile([1, Dh], BF16, tag="warm0_src")
    nc.vector.memset(warm0[:], 0.0)
    wps = aux.tile([Dh, Dh], F32, tag="aux", name=f"prewarm_{it}")
    nc.tensor.matmul(wps[:], warm0[:], warm0[:], start=True, stop=True)

    # ---------------- A = Wq^T @ Wk (fp32), then bf16 ----------------
    a_ps = scp.tile([Dh, Dh], F32, tag="sc", name=f"aps_{it}")
    nc.tensor.matmul(
        a_ps[:], wqk_sb[:, 0, :], wqk_sb[:, 1, :], start=True, stop=True
    )
    a_bf = persist.tile([Dh, Dh], BF16, tag="a_bf")
    nc.vector.tensor_copy(a_bf[:], a_ps[:])

    u_bf = [persist.tile([Dh, S], BF16, tag=f"u{i}", name=f"u{i}_{it}") for i in range(2)]

    def u_piece(j, t):
        def mm(tl):
            nc.tensor.matmul(
                tl[:], a_bf[:], qT[:, j, 512 * t : 512 * (t + 1)],
                start=True, stop=True,
            )

        def fin(tl):
            nc.vector.tensor_copy(u_bf[j % 2][:, 512 * t : 512 * (t + 1)], tl[:])

        return _Piece(
            lambda: aux.tile([Dh, 512], F32, tag="aux", name=f"u_{it}_{j}_{t}"),
            [mm], fin,
        )

    for t in range(2):
        u_ps = scp.tile([Dh, 512], F32, tag="sc", name=f"u0p_{it}_{t}")
        nc.tensor.matmul(
            u_ps[:], a_bf[:], qT[:, 0, 512 * t : 512 * (t + 1)],
            start=True, stop=True,
        )
        # parallel evacuation: DVE for t0, the still-idle ACT for t1
        if t == 0:
            nc.vector.tensor_copy(u_bf[0][:, 512 * t : 512 * (t + 1)], u_ps[:])
        else:
            nc.scalar.copy(u_bf[0][:, 512 * t : 512 * (t + 1)], u_ps[:])

    if _DEBUG:
        dbq = persist.tile([Dh, S], F32, tag="dbq")
        nc.vector.tensor_copy(dbq[:], qT[:, 0, :])
        nc.sync.dma_start(out=io["dbg_qt"][:, :], in_=dbq[:])
        dbu = persist.tile([Dh, S], F32, tag="dbu")
        nc.vector.tensor_copy(dbu[:], u_bf[0][:])
        nc.sync.dma_start(out=io["dbg_u"][:, :], in_=dbu[:])

    # ---------------- context staging / AllGather / out projection ------
    # Collective inputs: heads {0,1} full-S, heads {2,3} as one q-half plus
    # two q-quarters (the tail quarters AllGather separately so the last
    # one's chain is short).  Gathered slabs stage into single SBUF tiles
    # [128, 4(source core), cols] via one strided DMA each.
    in_cc_h = [
        dram.tile([2 * Dh, SH], BF16, name=f"incc_{it}_{hh}", tag=f"incc{hh}")
        for hh in range(2)
    ]
    in_cc2h0 = dram.tile([2 * Dh, SH], BF16, name=f"incc2h0_{it}", tag="incc2h0")
    in_cc2q = [
        dram.tile([2 * Dh, 512], BF16, name=f"incc2q_{it}_{qi}", tag=f"incc2q{qi}")
        for qi in range(2)
    ]
    ag_out0 = [
        dram.tile([512, SH], BF16, addr_space="Local", name=f"ag0_{it}_{hh}", tag=f"ag0{hh}")
        for hh in range(2)
    ]
    ag2h0 = dram.tile([512, SH], BF16, addr_space="Local", name=f"ag2h0_{it}", tag="ag2h0")
    ag2q = [
        dram.tile([512, 512], BF16, addr_space="Local", name=f"ag2q_{it}_{qi}", tag=f"ag2q{qi}")
        for qi in range(2)
    ]
    cch0 = [
        persist.tile([128, 4, SH], BF16, tag=f"cch0{hh}", name=f"cch0{hh}_{it}")
        for hh in range(2)
    ]
    cch_h0 = persist.tile([128, 4, SH], BF16, tag="cchh0", name=f"cchh0_{it}")
    cch_q = [
        persist.tile([128, 4, 512], BF16, tag=f"cchq{qi}", name=f"cchq{qi}_{it}")
        for qi in range(2)
    ]
    o_acc = [persist.tile([128, S], F32, tag=f"oacc{h}", name=f"oacc{h}_{it}") for h in range(2)]

    groups = [[0, 1, 2, 3], [4, 5, 6, 7]]

    def _ag(in_dram, out_dram, stage_tile, nsplit=1):
        if collective:
            nc.gpsimd.collective_compute(
                "AllGather", mybir.AluOpType.bypass, replica_groups=groups,
                ins=[in_dram[:, :].opt()], outs=[out_dram.opt()],
            )
        else:
            nc.sync.dma_start(out=out_dram[0:128, :], in_=in_dram[:, :])
        src = out_dram.rearrange("(r p) q -> p r q", p=128)
        if nsplit == 0:
            # one slice per source core so the consumer matmuls pipeline
            for r in range(4):
                nc.sync.dma_start(
                    out=stage_tile[:, r, :], in_=src[:, r, :]
                )
            return
        cols = stage_tile.shape[2] // nsplit
        for i in range(nsplit):
            nc.sync.dma_start(
                out=stage_tile[:, :, cols * i : cols * (i + 1)],
                in_=src[:, :, cols * i : cols * (i + 1)],
            )

    def emit_ag0(hh):
        _ag(in_cc_h[hh], ag_out0[hh], cch0[hh], nsplit=2)

    def emit_ag2h0():
        _ag(in_cc2h0, ag2h0, cch_h0)

    def emit_agq(qi):
        _ag(in_cc2q[qi], ag2q[qi], cch_q[qi], nsplit=0)

    tail_mode = {"on": False}

    def o_piece(round_, h, blk, tail=False):
        # blk: 512-wide query block of S (0..3); round 0 = even chunks
        # (heads 0,1 of each group) into o_acc, round 1 = odd chunks + bias.
        def mk_mm(r):
            def mm(tl):
                c8 = 2 * r + round_
                if round_ == 0:
                    src = cch0[blk // 2][:, r, 512 * (blk % 2) : 512 * (blk % 2 + 1)]
                elif blk < 2:
                    src = cch_h0[:, r, 512 * blk : 512 * (blk + 1)]
                else:
                    src = cch_q[blk - 2][:, r, :]
                nc.tensor.matmul(
                    tl[:], woT[:, c8, 128 * h : 128 * (h + 1)], src,
                    start=(r == 0), stop=(r == 3),
                )
            return mm

        def fin(tl):
            if round_ == 0:
                nc.vector.tensor_copy(o_acc[h][:, 512 * blk : 512 * (blk + 1)], tl[:])
            else:
                osb = osbp.tile([128, 512], F32, tag="osb", name=f"osb_{it}_{h}_{blk}")
                nc.vector.scalar_tensor_tensor(
                    osb[:], tl[:], bo_sb[:, h : h + 1],
                    o_acc[h][:, 512 * blk : 512 * (blk + 1)], ADD, ADD,
                )
                dma = (
                    (nc.scalar.dma_start if h == 1 else nc.sync.dma_start)
                    if (tail or tail_mode["on"])
                    else nc.sync.dma_start
                )
                dma(
                    out=outT[128 * h : 128 * (h + 1), 512 * blk : 512 * (blk + 1)],
                    in_=osb[:],
                )

        return _Piece(
            lambda: aux.tile([128, 512], F32, tag="aux", name=f"o_{it}_{round_}_{h}_{blk}"),
            [mk_mm(r) for r in range(4)], fin,
        )

    # ---------------- per-pass post-processing ----------------
    def pp_stage1(pi, j, q0, w, w2_t, tail=False):
        """Evacuate w2 (+row-sum row) to SBUF bf16; start 1/rowsum chain.
        In the tail the reciprocal leads (it gates the final mul); mid-kernel
        the evac leads (it frees the w2 psum banks for the next pass)."""
        w2sb = w2sbp.tile([Dh, w], BF16, tag="w2sb", name=f"w2sb_{it}_{pi}")
        rs_row = rsp.tile([1, w], F32, tag="rsrow", bufs=2, name=f"rsrow_{it}_{pi}")
        rsr = rsp.tile([1, w], F32, tag="rsr", bufs=2, name=f"rsr_{it}_{pi}")
        if tail:
            # ACT is idle after the last exp: it fetches the row-sum row
            # while DVE evacuates the values, shortening the serial chain
            nc.scalar.copy(rs_row[:], w2_t[Dh : Dh + 1, :])
            nc.vector.reciprocal_approx_fast(out=rsr[:], in_=rs_row[:])
            nc.vector.tensor_copy(w2sb[:], w2_t[0:Dh, :])
        else:
            nc.vector.tensor_copy(w2sb[:], w2_t[0:Dh, :])
            nc.vector.tensor_copy(rs_row[:], w2_t[Dh : Dh + 1, :])
            nc.vector.reciprocal_approx_fast(out=rsr[:], in_=rs_row[:])
        rs_b = rsp.tile([Dh, w], F32, tag="rsb", bufs=2, name=f"rsb_{it}_{pi}")
        nc.gpsimd.partition_broadcast(rs_b[:], rsr[:])
        return w2sb, rs_b

    def ctx_dma(j, q0, w, ctxT):
        """Write normalized context into the collective input buffers."""
        if j < 2:
            nc.sync.dma_start(
                out=in_cc_h[q0 // SH][Dh * j : Dh * (j + 1), :], in_=ctxT[:]
            )
            return
        row = Dh * (j - 2)
        for lo, hi, cont in (
            (0, SH, in_cc2h0),
            (SH, SH + 512, in_cc2q[0]),
            (SH + 512, S, in_cc2q[1]),
        ):
            a, b = max(q0, lo), min(q0 + w, hi)
            if a < b:
                nc.sync.dma_start(
                    out=cont[row : row + Dh, a - lo : b - lo],
                    in_=ctxT[:, a - q0 : b - q0],
                )

    def z_pieces(pi, j, q0, w, w2sb, rs_b, after):
        ctxT = ctxp.tile([Dh, w], BF16, tag="ctxT", name=f"ctxT_{it}_{pi}")
        n = w // 512
        done = [0]

        def mk_mm(t):
            def mm(tl):
                nc.tensor.matmul(
                    tl[:], wvT_sb[:], w2sb[0:Dh, 512 * t : 512 * (t + 1)],
                    start=True, stop=True,
                )
            return mm

        def mk_fin(t):
            def fin(tl):
                nc.vector.tensor_mul(
                    ctxT[:, 512 * t : 512 * (t + 1)], tl[:],
                    rs_b[:, 512 * t : 512 * (t + 1)],
                )
                done[0] += 1
                if done[0] == n:
                    if _DEBUG and pi == 0:
                        dbc = persist.tile([Dh, SH], F32, tag="dbc")
                        nc.vector.tensor_copy(dbc[:], ctxT[:])
                        nc.sync.dma_start(out=io["dbg_ctx"][:, :], in_=dbc[:])
                    ctx_dma(j, q0, w, ctxT)
                    if after is not None:
                        after()
            return fin

        return [
            _Piece(
                lambda t=t: aux.tile([Dh, 512], F32, tag="aux", name=f"z_{it}_{pi}_{t}"),
                [mk_mm(t)], mk_fin(t),
            )
            for t in range(n)
        ]

    # ---------------- main pass loop ----------------
    # (head, q-start, width); the last q-half runs as two quarter passes so
    # its AllGather/out-projection chain overlaps the final attention work.
    passes = [
        (0, 0, SH), (0, SH, SH),
        (1, 0, SH), (1, SH, SH),
        (2, 0, SH), (3, 0, SH), (2, SH, SH),
        (3, SH, 512), (3, SH + 512, 512),
    ]
    # AllGather trigger after the context of a given pass lands
    ag_after = {
        2: lambda: emit_ag0(0), 3: lambda: emit_ag0(1),
        5: emit_ag2h0, 7: lambda: emit_agq(0), 8: lambda: emit_agq(1),
    }
    u_after = {1: 1, 3: 2, 4: 3}   # pass index -> head whose u to prefetch

    fillers = _Fillers()
    state = {"pp": None, "z": None}

    def emit_pp(pi_now, tail=False):
        pj, pq0, pw, pw2, ppi = state["pp"]
        state["pp"] = None
        w2sb, rs_b = pp_stage1(ppi, pj, pq0, pw, pw2, tail=tail)
        if _DEBUG and ppi == 0:
            dbw = persist.tile([Dh, SH], F32, tag="dbw")
            nc.vector.tensor_copy(dbw[:], w2sb[:])
            nc.sync.dma_start(out=io["dbg_w2"][:, :], in_=dbw[:])
            nc.sync.dma_start(out=io["dbg_rs"][:, :], in_=rs_b[:])
        state["z"] = z_pieces(ppi, pj, pq0, pw, w2sb, rs_b, ag_after.get(ppi))

    def sched(pi, m):
        j, q0, w = passes[pi]
        if m == 2 and state["z"] is not None:
            for p in reversed(state["z"]):
                fillers.add(p, front=True)
            state["z"] = None
        if m == 5 and pi in u_after:
            fillers.finish_front()
        if m == 6 and pi in u_after:
            for t in reversed(range(4)):
                fillers.add(u_piece(u_after[pi], t), front=True)
        if pi == 0 and m == 0:
            for t in (2, 3):
                fillers.add(u_piece(0, t), front=True)
        if pi == 4 and m == 6:
            for h in range(2):
                for blk in (0, 1):
                    fillers.add(o_piece(0, h, blk))
        if pi == 5 and m == 6:
            for h in range(2):
                for blk in (2, 3):
                    fillers.add(o_piece(0, h, blk))
        if pi == 6 and m == 10:
            for h in range(2):
                fillers.add(o_piece(1, h, 0))
        if pi == 7 and m == 2:
            for h in range(2):
                fillers.add(o_piece(1, h, 1))

    # w2 emission schedule: chunk 2 opens the accumulation (start=True) at
    # unit 3, chunks 0/1 are deferred behind it, then one chunk per unit.
    # Scores are emitted one unit ahead (across pass boundaries) so the
    # next exp never queues behind the previous unit's w2 on PE.
    W2_ORDER_H = {3: [2, 0], 4: [1, 3]}
    W2_ORDER_H.update({u: [u - 1] for u in range(5, NK)})
    W2_ORDER_Q = {4: [3, 0], 5: [1, 4], 6: [2, 5]}
    W2_ORDER_Q.update({u: [u - 1] for u in range(7, NK)})

    units = [
        (pi, j, q0, w, m)
        for pi, (j, q0, w) in enumerate(passes)
        for m in range(NK)
    ]
    sc_tiles = {}
    w2_tiles = {}
    w2_thunks = {}

    def emit_sc(n):
        pi, j, q0, w, m = units[n]
        sc = scp.tile([128, w], F32, tag="sc", name=f"sc_{it}_{pi}_{m}")
        for u in range(w // 512):
            nc.tensor.matmul(
                sc[:, 512 * u : 512 * (u + 1)],
                kT[:, j, 128 * m : 128 * (m + 1)],
                u_bf[j % 2][:, q0 + 512 * u : q0 + 512 * (u + 1)],
                start=True, stop=True,
            )
        sc_tiles[n] = sc

    emit_sc(0)
    for n, (pi, j, q0, w, m) in enumerate(units):
        if n + 1 < len(units) and n + 1 not in sc_tiles:
            emit_sc(n + 1)
        if m >= NK - 2 and n + 2 < len(units):
            emit_sc(n + 2)
        if m == 0 and pi > 0:
            w2_thunks.pop((pi - 1, NK - 1))()  # previous pass's last w2
        if m == 0:
            w2_tiles[pi] = w2p.tile([Dh + 1, w], F32, tag="w2", name=f"w2_{it}_{pi}")
        order = W2_ORDER_H if w == SH else W2_ORDER_Q
        for w2m in order.get(m, ()):
            w2_thunks.pop((pi, w2m))()
        if m == 0 and state["pp"] is not None:
            emit_pp(pi)
        sched(pi, m)
        if 0 < m < NK - 1:
            fillers.pop_one()
        p_bf = ppool.tile([128, w], BF16, tag="p", name=f"p_{it}_{pi}_{m}")
        nc.scalar.activation(p_bf[:], sc_tiles.pop(n)[:], EXP, scale=0.125)
        if _DEBUG and pi == 0 and m == 0:
            dbp = persist.tile([128, SH], F32, tag="dbp")
            nc.vector.tensor_copy(dbp[:], p_bf[:])
            nc.sync.dma_start(out=io["dbg_p"][:, :], in_=dbp[:])

        def mk_w2(m_=m, p_=p_bf, pi_=pi, j_=j, w_=w):
            def go():
                for u in range(w_ // 512):
                    nc.tensor.matmul(
                        w2_tiles[pi_][:, 512 * u : 512 * (u + 1)],
                        vin[:, j_, m_, :], p_[:, 512 * u : 512 * (u + 1)],
                        start=(m_ == (2 if w_ == SH else 3)), stop=(m_ == NK - 1),
                    )
            return go

        w2_thunks[(pi, m)] = mk_w2()
        if m == NK - 1:
            state["pp"] = (j, q0, w, w2_tiles[pi], pi)

    # ---------------- tail ----------------
    w2_thunks.pop((len(passes) - 1, NK - 1))()
    emit_pp(None, tail=True)                   # rsr/evac/bcast first on DVE
    zs = state["z"]
    state["z"] = None
    for p in zs:                               # z + ctx DMA + final AllGather
        while not p.step():
            pass
    fillers.drain()                            # leftover half-0 round-1 pieces
    # quarter-0 out projection keeps PE busy while the final chain flies
    tailq0 = [o_piece(1, h, 2, tail=True) for h in range(2)]
    for r in range(4):
        for p in tailq0:
            p.step()
    # PE keep-warm chain across the AllGather latency: short PE->DVE->PE
    # round-trips every ~1.5us so the tensor engine's p-state never drops
    # before the final out-projection matmuls.
    warm_src = None
    for i in range(5):
        wt = aux.tile([Dh, 512], F32, tag="aux", name=f"warm_{it}_{i}")
        nc.tensor.matmul(
            wt[:], wvT_sb[:],
            warm_src if warm_src is not None else u_bf[1][:, 0:512],
            start=True, stop=True,
        )
        if i < 4:
            ws = persist.tile([Dh, 512], BF16, tag=f"warm{i}", name=f"wsc_{it}_{i}")
            nc.vector.tensor_copy(ws[:], wt[:])
            warm_src = ws[:]
    # final out-projection quarter: interleave the two pieces so their
    # accumulating matmuls pipeline behind the per-core staged slices
    tailp = [o_piece(1, h, 3, tail=True) for h in range(2)]
    for r in range(4):
        for p in tailp:
            p.step()


def _build(repeats=1, collective=True):
    key = (repeats, collective)
    if key in _CACHE:
        return _CACHE[key]
    ndev = N_CORES if collective else 1
    nc = bacc.Bacc("TRN2", target_bir_lowering=False, debug=False, num_devices=ndev)
    io = _declare_io(nc)
    with tile.TileContext(nc) as tc:
        for it in range(repeats):
            with contextlib.ExitStack() as es:
                _body(nc, tc, es, io, it, collective=collective)
    nc.compile()
    _CACHE[key] = nc
    return nc


def kernel(k_in, q_in, v_in, Wq, Wk, Wv, Wo, bo, _repeats=1, _results_hook=None):
    bf = ml_dtypes.bfloat16
    q_in = np.asarray(q_in, np.float32)
    k_in = np.asarray(k_in, np.float32)
    v_in = np.asarray(v_in, np.float32)
    Wq = np.asarray(Wq, np.float32)
    Wk = np.asarray(Wk, np.float32)
    Wv = np.asarray(Wv, np.float32)
    Wo = np.asarray(Wo, np.float32)
    bo = np.asarray(bo, np.float32)

    nc = _build(_repeats)

    wqk = np.ascontiguousarray(np.stack([Wq, Wk], axis=1))           # [64,2,64]
    wvT = np.ascontiguousarray(Wv.T.astype(bf))

    in_maps = []
    for c in range(N_CORES):
        b, q4 = c // 4, c % 4
        sl = slice(256 * q4, 256 * (q4 + 1))
        q_s = q_in[b, :, sl]                                         # [S, 256]
        k_s = k_in[b, :, sl]
        v_s = v_in[b, :, sl]
        vin = np.ones((128, HPC, NK, Dh + 1), np.float32)
        vin[:, :, :, :Dh] = v_s.reshape(NK, 128, HPC, Dh).transpose(1, 2, 0, 3)
        in_maps.append(
            {
                "qT": np.ascontiguousarray(
                    q_s.T.reshape(HPC, Dh, S).transpose(1, 0, 2).astype(bf)
                ),
                "kT": np.ascontiguousarray(
                    k_s.T.reshape(HPC, Dh, S).transpose(1, 0, 2).astype(bf)
                ),
                "vin": np.ascontiguousarray(vin.astype(bf)),
                "wqk": wqk,
                "wvT": wvT,
                "woT": np.ascontiguousarray(
                    Wo[sl, :].T.reshape(8, 128, EOUT).transpose(1, 0, 2).astype(bf)
                ),
                "bo2": np.ascontiguousarray(bo[sl].reshape(2, 128).T),
            }
        )

    res = run_bass_kernel_spmd(nc, in_maps, core_ids=list(range(N_CORES)))
    if _results_hook is not None:
        _results_hook(res)

    out = np.empty((B, S, E), dtype=np.float32)
    for c in range(N_CORES):
        b, q4 = c // 4, c % 4
        out[b, :, 256 * q4 : 256 * (q4 + 1)] = res.results[c]["outT"].T
    return out


# revision 69
# speedup vs baseline: 1.0337x; 1.0337x over previous
"""Multi-head attention kernel for Trainium2, SPMD over 8 NeuronCores.

Problem: B=2, S=2048, E=1024, H=16 heads, Dh=64.
  q = per-head q_in @ Wq.T (Wq shared across heads), same for k, v
  attn = softmax(q k^T / 8); ctx = attn @ v; out = concat(ctx) @ Wo.T + bo

Sharding: core c handles batch b=c//4 and heads 4*(c%4)..4*(c%4)+3
(head-parallel attention).  The out projection is sharded by e_out rows
(each core owns 256 rows of Wo), with AllGathers of the per-head context
over the 4 cores of each batch group in between.

All layout work happens host-side in kernel(): per-head transposed bf16
q/k ([Dh, head, S]), V repacked per key-chunk with a ones column riding
the contraction for the softmax row-sum, Wo pre-transposed into PE
stationary layout.  The device then does only the module's math:

  A = Wq^T Wk (fused q/k projection), u = A @ qT per head
  scores^T = kT_chunk^T u  ->  exp (ACT)  ->  w2 += vin_ones^T p (PSUM)
  ctx^T = (Wv @ w2) * (1/rowsum)    (normalization commutes with Wv)
  out = woT^T ctx_all + bo          (8 chunks, 2 AllGather rounds)

Schedule: a flat stream of (head, q-range) passes, each a 16-chunk key
loop, keeps ACT (exp) ~97-100% busy through the body; scores are emitted
one unit ahead (across pass boundaries) so exp never queues behind w2 on
the in-order PE queue, w2 of the first two chunks is deferred so the
single w2 PSUM buffer can turn around at pass boundaries, and a filler
queue trickles u/ctx/out-projection matmuls one per chunk into PE's
slack so its p-state stays at full clock.  The last q-half runs as two
quarter passes whose AllGather/out-projection chains overlap the final
attention; a short PE keep-warm chain bridges the tail's DMA-hop
latency.  PSUM: 2x scores double-buffer (4 banks) + w2 accumulator (2)
+ filler aux (2).
"""

import collections
import contextlib
import sys

sys.path.insert(0, "/opt/trn_rl_repo")

import ml_dtypes
import numpy as np

import concourse.bass as bass  # noqa: F401  (bass types via bacc)
import concourse.tile as tile
from concourse import bacc, mybir
from concourse.bass_utils import run_bass_kernel_spmd

B, S, E, H, Dh = 2, 2048, 1024, 16, 64
N_CORES = 8
HPC = 4              # heads per core
NK = S // 128        # 16 key chunks
EOUT = E // 4        # out-projection rows per core
SH = S // 2          # queries per (head, q-half) pass

F32 = mybir.dt.float32
BF16 = mybir.dt.bfloat16
EXP = mybir.ActivationFunctionType.Exp
ADD = mybir.AluOpType.add

_CACHE = {}
_DEBUG = False


def _declare_io(nc):
    io = {}
    io["qT"] = nc.dram_tensor("qT", [Dh, HPC, S], BF16, kind="ExternalInput").ap()
    io["kT"] = nc.dram_tensor("kT", [Dh, HPC, S], BF16, kind="ExternalInput").ap()
    io["vin"] = nc.dram_tensor(
        "vin", [128, HPC, NK, Dh + 1], BF16, kind="ExternalInput"
    ).ap()
    io["wqk"] = nc.dram_tensor("wqk", [Dh, 2, Dh], F32, kind="ExternalInput").ap()
    io["wvT"] = nc.dram_tensor("wvT", [Dh, Dh], BF16, kind="ExternalInput").ap()
    io["woT"] = nc.dram_tensor("woT", [128, 8, EOUT], BF16, kind="ExternalInput").ap()
    io["bo2"] = nc.dram_tensor("bo2", [128, 2], F32, kind="ExternalInput").ap()
    io["outT"] = nc.dram_tensor("outT", [EOUT, S], F32, kind="ExternalOutput").ap()
    if _DEBUG:
        for nm, shape in (
            ("dbg_qt", [Dh, S]), ("dbg_u", [Dh, S]), ("dbg_p", [128, SH]),
            ("dbg_w2", [Dh, SH]), ("dbg_rs", [Dh, SH]), ("dbg_ctx", [Dh, SH]),
        ):
            io[nm] = nc.dram_tensor(nm, shape, F32, kind="ExternalOutput").ap()
    return io


class _Piece:
    """A PE filler item: a few matmuls into one aux-psum tile plus a
    finishing (evacuation) op; emitted one matmul per attention unit."""

    def __init__(self, alloc, mms, fin):
        self.alloc = alloc
        self.mms = list(mms)
        self.fin = fin
        self.tile = None

    def step(self):
        if self.tile is None:
            self.tile = self.alloc()
        self.mms.pop(0)(self.tile)
        if not self.mms:
            if self.fin is not None:
                self.fin(self.tile)
            return True
        return False


class _Fillers:
    def __init__(self):
        self.q = collections.deque()

    def add(self, piece, front=False):
        (self.q.appendleft if front else self.q.append)(piece)

    def pop_one(self):
        if not self.q:
            return
        if self.q[0].step():
            self.q.popleft()

    def drain(self):
        while self.q:
            self.pop_one()

    def finish_front(self):
        """Run the front piece to completion so its aux slot frees."""
        if self.q and self.q[0].tile is not None:
            while not self.q[0].step():
                pass
            self.q.popleft()


def _body(nc, tc, es, io, it, collective=True):
    def pool(name, bufs, space="SBUF"):
        return es.enter_context(
            tc.tile_pool(name=f"{name}_{it}", bufs=bufs, space=space)
        )

    qTd, kTd, vind = io["qT"], io["kT"], io["vin"]
    wqkd, wvTd, woTd, bo2d, outT = (
        io["wqk"], io["wvT"], io["woT"], io["bo2"], io["outT"],
    )

    persist = pool("persist", 1)
    scp = pool("scp", 2, space="PSUM")    # 2x [128,2,512] f32 = 4 banks
    w2p = pool("w2p", 2, space="PSUM")    # 2x [65,512] f32 = 2 banks
    aux = pool("aux", 2, space="PSUM")    # 2x [128,512] f32 = 2 banks
    ppool = pool("ppool", 7)
    w2sbp = pool("w2sbp", 2)
    ctxp = pool("ctxp", 3)
    rsp = pool("rsp", 1)
    osbp = pool("osbp", 2)
    dram = pool("dram", 1, space="DRAM")

    # ---------------- input loads (host-prepped layouts) ----------------
    wqk_sb = persist.tile([Dh, 2, Dh], F32, tag="wqk")
    nc.sync.dma_start(out=wqk_sb[:], in_=wqkd[:, :, :])
    qT = persist.tile([Dh, HPC, S], BF16, tag="qT")
    nc.sync.dma_start(out=qT[:, 0, :], in_=qTd[:, 0, :])
    kT = persist.tile([Dh, HPC, S], BF16, tag="kT")
    nc.sync.dma_start(out=kT[:, 0, :], in_=kTd[:, 0, :])
    vin = persist.tile([128, HPC, NK, Dh + 1], BF16, tag="vin")
    nc.sync.dma_start(out=vin[:, 0, :, :], in_=vind[:, 0, :, :])
    wvT_sb = persist.tile([Dh, Dh], BF16, tag="wvT")
    nc.sync.dma_start(out=wvT_sb[:], in_=wvTd[:, :])
    nc.sync.dma_start(out=qT[:, 1:HPC, :], in_=qTd[:, 1:HPC, :])
    nc.sync.dma_start(out=kT[:, 1:HPC, :], in_=kTd[:, 1:HPC, :])
    nc.sync.dma_start(out=vin[:, 1:HPC, :, :], in_=vind[:, 1:HPC, :, :])
    woT = persist.tile([128, 8, EOUT], BF16, tag="woT")
    nc.sync.dma_start(out=woT[:], in_=woTd[:, :, :])
    bo_sb = persist.tile([128, 2], F32, tag="bo")
    nc.sync.dma_start(out=bo_sb[:], in_=bo2d[:, :])

    # ---------------- PE ramp warm-up ----------------
    # Two tiny matmuls on a zeroed tile start the tensor engine's p-state
    # ramp immediately so the first real matmuls run at speed.
    warm0 = persist.tile([1, Dh], BF16, tag="warm0_src")
    nc.vector.memset(warm0[:], 0.0)
    wps = aux.tile([Dh, Dh], F32, tag="aux", name=f"prewarm_{it}")
    nc.tensor.matmul(wps[:], warm0[:], warm0[:], start=True, stop=True)

    # ---------------- A = Wq^T @ Wk (fp32), then bf16 ----------------
    a_ps = scp.tile([Dh, Dh], F32, tag="sc", name=f"aps_{it}")
    nc.tensor.matmul(
        a_ps[:], wqk_sb[:, 0, :], wqk_sb[:, 1, :], start=True, stop=True
    )
    a_bf = persist.tile([Dh, Dh], BF16, tag="a_bf")
    nc.vector.tensor_copy(a_bf[:], a_ps[:])

    u_bf = [persist.tile([Dh, S], BF16, tag=f"u{i}", name=f"u{i}_{it}") for i in range(2)]

    def u_piece(j, t):
        def mm(tl):
            nc.tensor.matmul(
                tl[:], a_bf[:], qT[:, j, 512 * t : 512 * (t + 1)],
                start=True, stop=True,
            )

        def fin(tl):
            nc.vector.tensor_copy(u_bf[j % 2][:, 512 * t : 512 * (t + 1)], tl[:])

        return _Piece(
            lambda: aux.tile([Dh, 512], F32, tag="aux", name=f"u_{it}_{j}_{t}"),
            [mm], fin,
        )

    for t in range(2):
        u_ps = scp.tile([Dh, 512], F32, tag="sc", name=f"u0p_{it}_{t}")
        nc.tensor.matmul(
            u_ps[:], a_bf[:], qT[:, 0, 512 * t : 512 * (t + 1)],
            start=True, stop=True,
        )
        # parallel evacuation: DVE for t0, the still-idle ACT for t1
        if t == 0:
            nc.vector.tensor_copy(u_bf[0][:, 512 * t : 512 * (t + 1)], u_ps[:])
        else:
            nc.scalar.copy(u_bf[0][:, 512 * t : 512 * (t + 1)], u_ps[:])

    if _DEBUG:
        dbq = persist.tile([Dh, S], F32, tag="dbq")
        nc.vector.tensor_copy(dbq[:], qT[:, 0, :])
        nc.sync.dma_start(out=io["dbg_qt"][:, :], in_=dbq[:])
        dbu = persist.tile([Dh, S], F32, tag="dbu")
        nc.vector.tensor_copy(dbu[:], u_bf[0][:])
        nc.sync.dma_start(out=io["dbg_u"][:, :], in_=dbu[:])

    # ---------------- context staging / AllGather / out projection ------
    # Collective inputs: heads {0,1} full-S, heads {2,3} as one q-half plus
    # two q-quarters (the tail quarters AllGather separately so the last
    # one's chain is short).  Gathered slabs stage into single SBUF tiles
    # [128, 4(source core), cols] via one strided DMA each.
    in_cc_h = [
        dram.tile([2 * Dh, SH], BF16, name=f"incc_{it}_{hh}", tag=f"incc{hh}")
        for hh in range(2)
    ]
    in_cc2h0 = dram.tile([2 * Dh, SH], BF16, name=f"incc2h0_{it}", tag="incc2h0")
    in_cc2q = [
        dram.tile([2 * Dh, 512], BF16, name=f"incc2q_{it}_{qi}", tag=f"incc2q{qi}")
        for qi in range(2)
    ]
    ag_out0 = [
        dram.tile([512, SH], BF16, addr_space="Local", name=f"ag0_{it}_{hh}", tag=f"ag0{hh}")
        for hh in range(2)
    ]
    ag2h0 = dram.tile([512, SH], BF16, addr_space="Local", name=f"ag2h0_{it}", tag="ag2h0")
    ag2q = [
        dram.tile([512, 512], BF16, addr_space="Local", name=f"ag2q_{it}_{qi}", tag=f"ag2q{qi}")
        for qi in range(2)
    ]
    cch0 = [
        persist.tile([128, 4, SH], BF16, tag=f"cch0{hh}", name=f"cch0{hh}_{it}")
        for hh in range(2)
    ]
    cch_h0 = persist.tile([128, 4, SH], BF16, tag="cchh0", name=f"cchh0_{it}")
    cch_q = [
        persist.tile([128, 4, 512], BF16, tag=f"cchq{qi}", name=f"cchq{qi}_{it}")
        for qi in range(2)
    ]
    o_acc = [persist.tile([128, S], F32, tag=f"oacc{h}", name=f"oacc{h}_{it}") for h in range(2)]

    groups = [[0, 1, 2, 3], [4, 5, 6, 7]]

    def _ag(in_dram, out_dram, stage_tile, nsplit=1):
        if collective:
            nc.gpsimd.collective_compute(
                "AllGather", mybir.AluOpType.bypass, replica_groups=groups,
                ins=[in_dram[:, :].opt()], outs=[out_dram.opt()],
            )
        else:
            nc.sync.dma_start(out=out_dram[0:128, :], in_=in_dram[:, :])
        src = out_dram.rearrange("(r p) q -> p r q", p=128)
        if nsplit == 0:
            # one slice per source core so the consumer matmuls pipeline
            for r in range(4):
                nc.sync.dma_start(
                    out=stage_tile[:, r, :], in_=src[:, r, :]
                )
            return
        cols = stage_tile.shape[2] // nsplit
        for i in range(nsplit):
            nc.sync.dma_start(
                out=stage_tile[:, :, cols * i : cols * (i + 1)],
                in_=src[:, :, cols * i : cols * (i + 1)],
            )

    def emit_ag0(hh):
        _ag(in_cc_h[hh], ag_out0[hh], cch0[hh], nsplit=2)

    def emit_ag2h0():
        _ag(in_cc2h0, ag2h0, cch_h0)

    def emit_agq(qi):
        _ag(in_cc2q[qi], ag2q[qi], cch_q[qi], nsplit=0)

    tail_mode = {"on": False}

    def o_piece(round_, h, blk, tail=False):
        # blk: 512-wide query block of S (0..3); round 0 = even chunks
        # (heads 0,1 of each group) into o_acc, round 1 = odd chunks + bias.
        def mk_mm(r):
            def mm(tl):
                c8 = 2 * r + round_
                if round_ == 0:
                    src = cch0[blk // 2][:, r, 512 * (blk % 2) : 512 * (blk % 2 + 1)]
                elif blk < 2:
                    src = cch_h0[:, r, 512 * blk : 512 * (blk + 1)]
                else:
                    src = cch_q[blk - 2][:, r, :]
                nc.tensor.matmul(
                    tl[:], woT[:, c8, 128 * h : 128 * (h + 1)], src,
                    start=(r == 0), stop=(r == 3),
                )
            return mm

        def fin(tl):
            if round_ == 0:
                nc.vector.tensor_copy(o_acc[h][:, 512 * blk : 512 * (blk + 1)], tl[:])
            else:
                osb = osbp.tile([128, 512], F32, tag="osb", name=f"osb_{it}_{h}_{blk}")
                nc.vector.scalar_tensor_tensor(
                    osb[:], tl[:], bo_sb[:, h : h + 1],
                    o_acc[h][:, 512 * blk : 512 * (blk + 1)], ADD, ADD,
                )
                dma = (
                    (nc.scalar.dma_start if h == 1 else nc.sync.dma_start)
                    if (tail or tail_mode["on"])
                    else nc.sync.dma_start
                )
                dma(
                    out=outT[128 * h : 128 * (h + 1), 512 * blk : 512 * (blk + 1)],
                    in_=osb[:],
                )

        return _Piece(
            lambda: aux.tile([128, 512], F32, tag="aux", name=f"o_{it}_{round_}_{h}_{blk}"),
            [mk_mm(r) for r in range(4)], fin,
        )

    # ---------------- per-pass post-processing ----------------
    def pp_stage1(pi, j, q0, w, w2_t, tail=False):
        """Evacuate w2 (+row-sum row) to SBUF bf16; start 1/rowsum chain.
        In the tail the reciprocal leads (it gates the final mul); mid-kernel
        the evac leads (it frees the w2 psum banks for the next pass)."""
        w2sb = w2sbp.tile([Dh, w], BF16, tag="w2sb", name=f"w2sb_{it}_{pi}")
        rs_row = rsp.tile([1, w], F32, tag="rsrow", bufs=2, name=f"rsrow_{it}_{pi}")
        rsr = rsp.tile([1, w], F32, tag="rsr", bufs=2, name=f"rsr_{it}_{pi}")
        if tail:
            # ACT is idle after the last exp: it fetches the row-sum row
            # while DVE evacuates the values, shortening the serial chain
            nc.scalar.copy(rs_row[:], w2_t[Dh : Dh + 1, :])
            nc.vector.reciprocal_approx_fast(out=rsr[:], in_=rs_row[:])
            nc.vector.tensor_copy(w2sb[:], w2_t[0:Dh, :])
        else:
            nc.vector.tensor_copy(w2sb[:], w2_t[0:Dh, :])
            nc.vector.tensor_copy(rs_row[:], w2_t[Dh : Dh + 1, :])
            nc.vector.reciprocal_approx_fast(out=rsr[:], in_=rs_row[:])
        rs_b = rsp.tile([Dh, w], F32, tag="rsb", bufs=2, name=f"rsb_{it}_{pi}")
        nc.gpsimd.partition_broadcast(rs_b[:], rsr[:])
        return w2sb, rs_b

    def ctx_dma(j, q0, w, ctxT):
        """Write normalized context into the collective input buffers."""
        if j < 2:
            c0 = q0 % SH
            nc.sync.dma_start(
                out=in_cc_h[q0 // SH][Dh * j : Dh * (j + 1), c0 : c0 + w],
                in_=ctxT[:],
            )
            return
        row = Dh * (j - 2)
        for lo, hi, cont in (
            (0, SH, in_cc2h0),
            (SH, SH + 512, in_cc2q[0]),
            (SH + 512, S, in_cc2q[1]),
        ):
            a, b = max(q0, lo), min(q0 + w, hi)
            if a < b:
                nc.sync.dma_start(
                    out=cont[row : row + Dh, a - lo : b - lo],
                    in_=ctxT[:, a - q0 : b - q0],
                )

    def z_pieces(pi, j, q0, w, w2sb, rs_b, after):
        ctxT = ctxp.tile([Dh, w], BF16, tag="ctxT", name=f"ctxT_{it}_{pi}")
        n = w // 512
        done = [0]

        def mk_mm(t):
            def mm(tl):
                nc.tensor.matmul(
                    tl[:], wvT_sb[:], w2sb[0:Dh, 512 * t : 512 * (t + 1)],
                    start=True, stop=True,
                )
            return mm

        def mk_fin(t):
            def fin(tl):
                nc.vector.tensor_mul(
                    ctxT[:, 512 * t : 512 * (t + 1)], tl[:],
                    rs_b[:, 512 * t : 512 * (t + 1)],
                )
                done[0] += 1
                if done[0] == n:
                    if _DEBUG and pi == 0:
                        dbc = persist.tile([Dh, SH], F32, tag="dbc")
                        nc.vector.tensor_copy(dbc[:], ctxT[:])
                        nc.sync.dma_start(out=io["dbg_ctx"][:, :], in_=dbc[:])
                    ctx_dma(j, q0, w, ctxT)
                    if after is not None:
                        after()
            return fin

        return [
            _Piece(
                lambda t=t: aux.tile([Dh, 512], F32, tag="aux", name=f"z_{it}_{pi}_{t}"),
                [mk_mm(t)], mk_fin(t),
            )
            for t in range(n)
        ]

    # ---------------- main pass loop ----------------
    # (head, q-start, width); the last q-half runs as two quarter passes so
    # its AllGather/out-projection chain overlaps the final attention work.
    Q = 512
    passes = (
        [(0, Q * i, Q) for i in range(4)]
        + [(1, Q * i, Q) for i in range(4)]
        + [(2, 0, Q), (2, Q, Q), (3, 0, Q), (3, Q, Q)]
        + [(2, 2 * Q, Q), (2, 3 * Q, Q), (3, 2 * Q, Q), (3, 3 * Q, Q)]
    )
    # AllGather trigger after the context of a given pass lands
    ag_after = {
        5: lambda: emit_ag0(0), 7: lambda: emit_ag0(1),
        11: emit_ag2h0, 14: lambda: emit_agq(0), 15: lambda: emit_agq(1),
    }
    u_after = {2: 1, 6: 2, 9: 3}   # pass index -> head whose u to prefetch

    fillers = _Fillers()
    state = {"pp": None, "z": None}

    def emit_pp(pi_now, tail=False):
        pj, pq0, pw, pw2, ppi = state["pp"]
        state["pp"] = None
        w2sb, rs_b = pp_stage1(ppi, pj, pq0, pw, pw2, tail=tail)
        if _DEBUG and ppi == 0:
            dbw = persist.tile([Dh, SH], F32, tag="dbw")
            nc.vector.tensor_copy(dbw[:], w2sb[:])
            nc.sync.dma_start(out=io["dbg_w2"][:, :], in_=dbw[:])
            nc.sync.dma_start(out=io["dbg_rs"][:, :], in_=rs_b[:])
        state["z"] = z_pieces(ppi, pj, pq0, pw, w2sb, rs_b, ag_after.get(ppi))

    def sched(pi, m):
        j, q0, w = passes[pi]
        if m == 2 and state["z"] is not None:
            for p in reversed(state["z"]):
                fillers.add(p, front=True)
            state["z"] = None
        if m == 2 and pi in u_after:
            fillers.finish_front()
        if m == 3 and pi in u_after:
            for t in reversed(range(4)):
                fillers.add(u_piece(u_after[pi], t), front=True)
        if pi == 0 and m == 0:
            for t in (2, 3):
                fillers.add(u_piece(0, t), front=True)
        if pi == 8 and m == 2:
            for h in range(2):
                for blk in (0, 1):
                    fillers.add(o_piece(0, h, blk))
        if pi == 10 and m == 2:
            for h in range(2):
                for blk in (2, 3):
                    fillers.add(o_piece(0, h, blk))
        if pi == 13 and m == 2:
            for h in range(2):
                fillers.add(o_piece(1, h, 0))
        if pi == 14 and m == 1:
            for h in range(2):
                fillers.add(o_piece(1, h, 1))

    # Each unit covers TWO key chunks for a 512-wide q-range: the scores
    # land side by side in one [128, 2, 512] psum tile so a single
    # [128,1024]-wide exp instruction serves both chunks (amortizing ACT's
    # per-instruction overhead).  Scores are emitted one unit ahead (two at
    # pass boundaries); w2 is double-buffered so boundaries need no
    # turnaround slack.
    NK2 = NK // 2
    units = [
        (pi, j, q0, w, m2)
        for pi, (j, q0, w) in enumerate(passes)
        for m2 in range(NK2)
    ]
    sc_tiles = {}
    w2_tiles = {}
    w2_thunks = {}

    def emit_sc(n):
        pi, j, q0, w, m2 = units[n]
        sc = scp.tile([128, 2, 512], F32, tag="sc", name=f"sc_{it}_{pi}_{m2}")
        for c in range(2):
            mm = 2 * m2 + c
            nc.tensor.matmul(
                sc[:, c, :],
                kT[:, j, 128 * mm : 128 * (mm + 1)],
                u_bf[j % 2][:, q0 : q0 + 512],
                start=True, stop=True,
            )
        sc_tiles[n] = sc

    emit_sc(0)
    for n, (pi, j, q0, w, m2) in enumerate(units):
        if n + 1 < len(units) and n + 1 not in sc_tiles:
            emit_sc(n + 1)
        if m2 >= NK2 - 2 and n + 2 < len(units):
            emit_sc(n + 2)
        if m2 == 0 and pi > 0:
            w2_thunks.pop((pi - 1, NK2 - 1))()  # previous pass's last w2
        if m2 == 0:
            w2_tiles[pi] = w2p.tile([Dh + 1, 512], F32, tag="w2", name=f"w2_{it}_{pi}")
        if m2 > 0:
            w2_thunks.pop((pi, m2 - 1))()
        if m2 == 0 and state["pp"] is not None:
            emit_pp(pi)
        sched(pi, m2)
        if 0 < m2 < NK2 - 1:
            fillers.pop_one()
        p_bf = ppool.tile([128, 2, 512], BF16, tag="p", name=f"p_{it}_{pi}_{m2}")
        nc.scalar.activation(p_bf[:], sc_tiles.pop(n)[:], EXP, scale=0.125)

        def mk_w2(m2_=m2, p_=p_bf, pi_=pi, j_=j):
            def go():
                for c in range(2):
                    mm = 2 * m2_ + c
                    nc.tensor.matmul(
                        w2_tiles[pi_][:, :],
                        vin[:, j_, mm, :], p_[:, c, :],
                        start=(m2_ == 0 and c == 0),
                        stop=(m2_ == NK2 - 1 and c == 1),
                    )
            return go

        w2_thunks[(pi, m2)] = mk_w2()
        if m2 == NK2 - 1:
            state["pp"] = (j, q0, w, w2_tiles[pi], pi)

    # ---------------- tail ----------------
    w2_thunks.pop((len(passes) - 1, NK2 - 1))()
    emit_pp(None, tail=True)                   # rsr/evac/bcast first on DVE
    zs = state["z"]
    state["z"] = None
    for p in zs:                               # z + ctx DMA + final AllGather
        while not p.step():
            pass
    fillers.drain()                            # leftover half-0 round-1 pieces
    # quarter-0 out projection keeps PE busy while the final chain flies
    tailq0 = [o_piece(1, h, 2, tail=True) for h in range(2)]
    for r in range(4):
        for p in tailq0:
            p.step()
    # PE keep-warm chain across the AllGather latency: short PE->DVE->PE
    # round-trips every ~1.5us so the tensor engine's p-state never drops
    # before the final out-projection matmuls.
    warm_src = None
    for i in range(5):
        wt = aux.tile([Dh, 512], F32, tag="aux", name=f"warm_{it}_{i}")
        nc.tensor.matmul(
            wt[:], wvT_sb[:],
            warm_src if warm_src is not None else u_bf[1][:, 0:512],
            start=True, stop=True,
        )
        if i < 4:
            ws = persist.tile([Dh, 512], BF16, tag=f"warm{i}", name=f"wsc_{it}_{i}")
            nc.vector.tensor_copy(ws[:], wt[:])
            warm_src = ws[:]
    # final out-projection quarter: interleave the two pieces so their
    # accumulating matmuls pipeline behind the per-core staged slices
    tailp = [o_piece(1, h, 3, tail=True) for h in range(2)]
    for r in range(4):
        for p in tailp:
            p.step()


def _build(repeats=1, collective=True):
    key = (repeats, collective)
    if key in _CACHE:
        return _CACHE[key]
    ndev = N_CORES if collective else 1
    nc = bacc.Bacc("TRN2", target_bir_lowering=False, debug=False, num_devices=ndev)
    io = _declare_io(nc)
    with tile.TileContext(nc) as tc:
        for it in range(repeats):
            with contextlib.ExitStack() as es:
                _body(nc, tc, es, io, it, collective=collective)
    nc.compile()
    _CACHE[key] = nc
    return nc


def kernel(k_in, q_in, v_in, Wq, Wk, Wv, Wo, bo, _repeats=1, _results_hook=None):
    bf = ml_dtypes.bfloat16
    q_in = np.asarray(q_in, np.float32)
    k_in = np.asarray(k_in, np.float32)
    v_in = np.asarray(v_in, np.float32)
    Wq = np.asarray(Wq, np.float32)
    Wk = np.asarray(Wk, np.float32)
    Wv = np.asarray(Wv, np.float32)
    Wo = np.asarray(Wo, np.float32)
    bo = np.asarray(bo, np.float32)

    nc = _build(_repeats)

    wqk = np.ascontiguousarray(np.stack([Wq, Wk], axis=1))           # [64,2,64]
    wvT = np.ascontiguousarray(Wv.T.astype(bf))

    in_maps = []
    for c in range(N_CORES):
        b, q4 = c // 4, c % 4
        sl = slice(256 * q4, 256 * (q4 + 1))
        q_s = q_in[b, :, sl]                                         # [S, 256]
        k_s = k_in[b, :, sl]
        v_s = v_in[b, :, sl]
        vin = np.ones((128, HPC, NK, Dh + 1), np.float32)
        vin[:, :, :, :Dh] = v_s.reshape(NK, 128, HPC, Dh).transpose(1, 2, 0, 3)
        in_maps.append(
            {
                "qT": np.ascontiguousarray(
                    q_s.T.reshape(HPC, Dh, S).transpose(1, 0, 2).astype(bf)
                ),
                "kT": np.ascontiguousarray(
                    k_s.T.reshape(HPC, Dh, S).transpose(1, 0, 2).astype(bf)
                ),
                "vin": np.ascontiguousarray(vin.astype(bf)),
                "wqk": wqk,
                "wvT": wvT,
                "woT": np.ascontiguousarray(
                    Wo[sl, :].T.reshape(8, 128, EOUT).transpose(1, 0, 2).astype(bf)
                ),
                "bo2": np.ascontiguousarray(bo[sl].reshape(2, 128).T),
            }
        )

    res = run_bass_kernel_spmd(nc, in_maps, core_ids=list(range(N_CORES)))
    if _results_hook is not None:
        _results_hook(res)

    out = np.empty((B, S, E), dtype=np.float32)
    for c in range(N_CORES):
        b, q4 = c // 4, c % 4
        out[b, :, 256 * q4 : 256 * (q4 + 1)] = res.results[c]["outT"].T
    return out


# revision 75
# speedup vs baseline: 1.0416x; 1.0077x over previous
"""Multi-head attention kernel for Trainium2, SPMD over 8 NeuronCores.

Problem: B=2, S=2048, E=1024, H=16 heads, Dh=64.
  q = per-head q_in @ Wq.T (Wq shared across heads), same for k, v
  attn = softmax(q k^T / 8); ctx = attn @ v; out = concat(ctx) @ Wo.T + bo

Sharding: core c handles batch b=c//4 and heads 4*(c%4)..4*(c%4)+3
(head-parallel attention).  The out projection is sharded by e_out rows
(each core owns 256 rows of Wo), with AllGathers of the per-head context
over the 4 cores of each batch group in between.

All layout work happens host-side in kernel(): per-head transposed bf16
q/k ([Dh, head, S]), V repacked per key-chunk with a ones column riding
the contraction for the softmax row-sum, Wo pre-transposed into PE
stationary layout.  The device then does only the module's math:

  A = Wq^T Wk (fused q/k projection), u = A @ qT per head
  scores^T = kT_chunk^T u  ->  exp (ACT)  ->  w2 += vin_ones^T p (PSUM)
  ctx^T = (Wv @ w2) * (1/rowsum)    (normalization commutes with Wv)
  out = woT^T ctx_all + bo          (8 chunks, 2 AllGather rounds)

Schedule: a flat stream of 16 (head, 512-wide q-range) passes.  Each
unit covers TWO key chunks whose scores land side by side in one
[128,2,512] psum tile, so a single [128,1024]-wide exp instruction
serves both chunks (amortizing ACT's per-instruction overhead); ACT
runs ~97-100% busy through the body.  Scores are emitted one unit ahead
(two at pass boundaries) so exp never queues behind w2 on the in-order
PE queue; the w2 accumulator is double-buffered so pass boundaries need
no turnaround slack; a filler queue trickles u/ctx/out-projection
matmuls one per unit into PE's headroom so its p-state stays at full
clock.  Heads interleave as h2,h3 half-wise so the heads{2,3}
AllGathers fire early, and the final two quarter passes' gather/
out-projection chains overlap the last attention work; a short PE
keep-warm chain bridges the tail's DMA-hop latency.  PSUM: 2x paired
score tiles (4 banks) + 2x w2 (2) + filler aux (2).
"""

import collections
import contextlib
import sys

sys.path.insert(0, "/opt/trn_rl_repo")

import ml_dtypes
import numpy as np

import concourse.bass as bass  # noqa: F401  (bass types via bacc)
import concourse.tile as tile
from concourse import bacc, mybir
from concourse.bass_utils import run_bass_kernel_spmd

B, S, E, H, Dh = 2, 2048, 1024, 16, 64
N_CORES = 8
HPC = 4              # heads per core
NK = S // 128        # 16 key chunks
EOUT = E // 4        # out-projection rows per core
SH = S // 2          # queries per (head, q-half) pass

F32 = mybir.dt.float32
BF16 = mybir.dt.bfloat16
EXP = mybir.ActivationFunctionType.Exp
ADD = mybir.AluOpType.add

_CACHE = {}
_DEBUG = False


def _declare_io(nc):
    io = {}
    io["qT"] = nc.dram_tensor("qT", [Dh, HPC, S], BF16, kind="ExternalInput").ap()
    io["kT"] = nc.dram_tensor("kT", [Dh, HPC, S], BF16, kind="ExternalInput").ap()
    io["vin"] = nc.dram_tensor(
        "vin", [128, HPC, NK, Dh + 1], BF16, kind="ExternalInput"
    ).ap()
    io["wqk"] = nc.dram_tensor("wqk", [Dh, 2, Dh], F32, kind="ExternalInput").ap()
    io["wvT"] = nc.dram_tensor("wvT", [Dh, Dh], BF16, kind="ExternalInput").ap()
    io["woT"] = nc.dram_tensor("woT", [128, 8, EOUT], BF16, kind="ExternalInput").ap()
    io["bo2"] = nc.dram_tensor("bo2", [128, 2], F32, kind="ExternalInput").ap()
    io["outT"] = nc.dram_tensor("outT", [EOUT, S], F32, kind="ExternalOutput").ap()
    if _DEBUG:
        for nm, shape in (
            ("dbg_qt", [Dh, S]), ("dbg_u", [Dh, S]), ("dbg_p", [128, SH]),
            ("dbg_w2", [Dh, SH]), ("dbg_rs", [Dh, SH]), ("dbg_ctx", [Dh, SH]),
        ):
            io[nm] = nc.dram_tensor(nm, shape, F32, kind="ExternalOutput").ap()
    return io


class _Piece:
    """A PE filler item: a few matmuls into one aux-psum tile plus a
    finishing (evacuation) op; emitted one matmul per attention unit."""

    def __init__(self, alloc, mms, fin):
        self.alloc = alloc
        self.mms = list(mms)
        self.fin = fin
        self.tile = None

    def step(self):
        if self.tile is None:
            self.tile = self.alloc()
        self.mms.pop(0)(self.tile)
        if not self.mms:
            if self.fin is not None:
                self.fin(self.tile)
            return True
        return False


class _Fillers:
    def __init__(self):
        self.q = collections.deque()

    def add(self, piece, front=False):
        (self.q.appendleft if front else self.q.append)(piece)

    def pop_one(self):
        if not self.q:
            return
        if self.q[0].step():
            self.q.popleft()

    def drain(self):
        while self.q:
            self.pop_one()

    def finish_front(self):
        """Run the front piece to completion so its aux slot frees."""
        if self.q and self.q[0].tile is not None:
            while not self.q[0].step():
                pass
            self.q.popleft()


def _body(nc, tc, es, io, it, collective=True):
    def pool(name, bufs, space="SBUF"):
        return es.enter_context(
            tc.tile_pool(name=f"{name}_{it}", bufs=bufs, space=space)
        )

    qTd, kTd, vind = io["qT"], io["kT"], io["vin"]
    wqkd, wvTd, woTd, bo2d, outT = (
        io["wqk"], io["wvT"], io["woT"], io["bo2"], io["outT"],
    )

    persist = pool("persist", 1)
    scp = pool("scp", 2, space="PSUM")    # 2x [128,2,512] f32 = 4 banks
    w2p = pool("w2p", 2, space="PSUM")    # 2x [65,512] f32 = 2 banks
    aux = pool("aux", 2, space="PSUM")    # 2x [128,512] f32 = 2 banks
    ppool = pool("ppool", 7)
    w2sbp = pool("w2sbp", 2)
    ctxp = pool("ctxp", 3)
    rsp = pool("rsp", 1)
    osbp = pool("osbp", 2)
    dram = pool("dram", 1, space="DRAM")

    # ---------------- input loads (host-prepped layouts) ----------------
    wqk_sb = persist.tile([Dh, 2, Dh], F32, tag="wqk")
    nc.sync.dma_start(out=wqk_sb[:], in_=wqkd[:, :, :])
    qT = persist.tile([Dh, HPC, S], BF16, tag="qT")
    nc.sync.dma_start(out=qT[:, 0, :], in_=qTd[:, 0, :])
    kT = persist.tile([Dh, HPC, S], BF16, tag="kT")
    nc.sync.dma_start(out=kT[:, 0, :], in_=kTd[:, 0, :])
    vin = persist.tile([128, HPC, NK, Dh + 1], BF16, tag="vin")
    nc.sync.dma_start(out=vin[:, 0, :, :], in_=vind[:, 0, :, :])
    wvT_sb = persist.tile([Dh, Dh], BF16, tag="wvT")
    nc.sync.dma_start(out=wvT_sb[:], in_=wvTd[:, :])
    nc.sync.dma_start(out=qT[:, 1:HPC, :], in_=qTd[:, 1:HPC, :])
    nc.sync.dma_start(out=kT[:, 1:HPC, :], in_=kTd[:, 1:HPC, :])
    nc.sync.dma_start(out=vin[:, 1:HPC, :, :], in_=vind[:, 1:HPC, :, :])
    woT = persist.tile([128, 8, EOUT], BF16, tag="woT")
    nc.sync.dma_start(out=woT[:], in_=woTd[:, :, :])
    bo_sb = persist.tile([128, 2], F32, tag="bo")
    nc.sync.dma_start(out=bo_sb[:], in_=bo2d[:, :])

    # ---------------- PE ramp warm-up ----------------
    # Two tiny matmuls on a zeroed tile start the tensor engine's p-state
    # ramp immediately so the first real matmuls run at speed.
    warm0 = persist.tile([1, Dh], BF16, tag="warm0_src")
    nc.vector.memset(warm0[:], 0.0)
    wps = aux.tile([Dh, Dh], F32, tag="aux", name=f"prewarm_{it}")
    nc.tensor.matmul(wps[:], warm0[:], warm0[:], start=True, stop=True)

    # ---------------- A = Wq^T @ Wk (fp32), then bf16 ----------------
    a_ps = scp.tile([Dh, Dh], F32, tag="sc", name=f"aps_{it}")
    nc.tensor.matmul(
        a_ps[:], wqk_sb[:, 0, :], wqk_sb[:, 1, :], start=True, stop=True
    )
    a_bf = persist.tile([Dh, Dh], BF16, tag="a_bf")
    nc.vector.tensor_copy(a_bf[:], a_ps[:])

    u_bf = [persist.tile([Dh, S], BF16, tag=f"u{i}", name=f"u{i}_{it}") for i in range(2)]

    def u_piece(j, t):
        def mm(tl):
            nc.tensor.matmul(
                tl[:], a_bf[:], qT[:, j, 512 * t : 512 * (t + 1)],
                start=True, stop=True,
            )

        def fin(tl):
            nc.vector.tensor_copy(u_bf[j % 2][:, 512 * t : 512 * (t + 1)], tl[:])

        return _Piece(
            lambda: aux.tile([Dh, 512], F32, tag="aux", name=f"u_{it}_{j}_{t}"),
            [mm], fin,
        )

    for t in range(2):
        u_ps = scp.tile([Dh, 512], F32, tag="sc", name=f"u0p_{it}_{t}")
        nc.tensor.matmul(
            u_ps[:], a_bf[:], qT[:, 0, 512 * t : 512 * (t + 1)],
            start=True, stop=True,
        )
        # parallel evacuation: DVE for t0, the still-idle ACT for t1
        if t == 0:
            nc.vector.tensor_copy(u_bf[0][:, 512 * t : 512 * (t + 1)], u_ps[:])
        else:
            nc.scalar.copy(u_bf[0][:, 512 * t : 512 * (t + 1)], u_ps[:])

    if _DEBUG:
        dbq = persist.tile([Dh, S], F32, tag="dbq")
        nc.vector.tensor_copy(dbq[:], qT[:, 0, :])
        nc.sync.dma_start(out=io["dbg_qt"][:, :], in_=dbq[:])
        dbu = persist.tile([Dh, S], F32, tag="dbu")
        nc.vector.tensor_copy(dbu[:], u_bf[0][:])
        nc.sync.dma_start(out=io["dbg_u"][:, :], in_=dbu[:])

    # ---------------- context staging / AllGather / out projection ------
    # Collective inputs: heads {0,1} full-S, heads {2,3} as one q-half plus
    # two q-quarters (the tail quarters AllGather separately so the last
    # one's chain is short).  Gathered slabs stage into single SBUF tiles
    # [128, 4(source core), cols] via one strided DMA each.
    in_cc_h = [
        dram.tile([2 * Dh, SH], BF16, name=f"incc_{it}_{hh}", tag=f"incc{hh}")
        for hh in range(2)
    ]
    in_cc2h0 = dram.tile([2 * Dh, SH], BF16, name=f"incc2h0_{it}", tag="incc2h0")
    in_cc2q = [
        dram.tile([2 * Dh, 512], BF16, name=f"incc2q_{it}_{qi}", tag=f"incc2q{qi}")
        for qi in range(2)
    ]
    ag_out0 = [
        dram.tile([512, SH], BF16, addr_space="Local", name=f"ag0_{it}_{hh}", tag=f"ag0{hh}")
        for hh in range(2)
    ]
    ag2h0 = dram.tile([512, SH], BF16, addr_space="Local", name=f"ag2h0_{it}", tag="ag2h0")
    ag2q = [
        dram.tile([512, 512], BF16, addr_space="Local", name=f"ag2q_{it}_{qi}", tag=f"ag2q{qi}")
        for qi in range(2)
    ]
    cch0 = [
        persist.tile([128, 4, SH], BF16, tag=f"cch0{hh}", name=f"cch0{hh}_{it}")
        for hh in range(2)
    ]
    cch_h0 = persist.tile([128, 4, SH], BF16, tag="cchh0", name=f"cchh0_{it}")
    cch_q = [
        persist.tile([128, 4, 512], BF16, tag=f"cchq{qi}", name=f"cchq{qi}_{it}")
        for qi in range(2)
    ]
    o_acc = [persist.tile([128, S], F32, tag=f"oacc{h}", name=f"oacc{h}_{it}") for h in range(2)]

    groups = [[0, 1, 2, 3], [4, 5, 6, 7]]

    def _ag(in_dram, out_dram, stage_tile, nsplit=1):
        if collective:
            nc.gpsimd.collective_compute(
                "AllGather", mybir.AluOpType.bypass, replica_groups=groups,
                ins=[in_dram[:, :].opt()], outs=[out_dram.opt()],
            )
        else:
            nc.sync.dma_start(out=out_dram[0:128, :], in_=in_dram[:, :])
        src = out_dram.rearrange("(r p) q -> p r q", p=128)
        if nsplit == 0:
            # source-core pairs: consumers pipeline after each half-slab
            for r in (0, 2):
                nc.sync.dma_start(
                    out=stage_tile[:, r : r + 2, :], in_=src[:, r : r + 2, :]
                )
            return
        cols = stage_tile.shape[2] // nsplit
        for i in range(nsplit):
            nc.sync.dma_start(
                out=stage_tile[:, :, cols * i : cols * (i + 1)],
                in_=src[:, :, cols * i : cols * (i + 1)],
            )

    def emit_ag0(hh):
        _ag(in_cc_h[hh], ag_out0[hh], cch0[hh], nsplit=2)

    def emit_ag2h0():
        _ag(in_cc2h0, ag2h0, cch_h0)

    def emit_agq(qi):
        _ag(in_cc2q[qi], ag2q[qi], cch_q[qi], nsplit=0)

    tail_mode = {"on": False}

    def o_piece(round_, h, blk, tail=False):
        # blk: 512-wide query block of S (0..3); round 0 = even chunks
        # (heads 0,1 of each group) into o_acc, round 1 = odd chunks + bias.
        def mk_mm(r):
            def mm(tl):
                c8 = 2 * r + round_
                if round_ == 0:
                    src = cch0[blk // 2][:, r, 512 * (blk % 2) : 512 * (blk % 2 + 1)]
                elif blk < 2:
                    src = cch_h0[:, r, 512 * blk : 512 * (blk + 1)]
                else:
                    src = cch_q[blk - 2][:, r, :]
                nc.tensor.matmul(
                    tl[:], woT[:, c8, 128 * h : 128 * (h + 1)], src,
                    start=(r == 0), stop=(r == 3),
                )
            return mm

        def fin(tl):
            if round_ == 0:
                nc.vector.tensor_copy(o_acc[h][:, 512 * blk : 512 * (blk + 1)], tl[:])
            else:
                osb = osbp.tile([128, 512], F32, tag="osb", name=f"osb_{it}_{h}_{blk}")
                nc.vector.scalar_tensor_tensor(
                    osb[:], tl[:], bo_sb[:, h : h + 1],
                    o_acc[h][:, 512 * blk : 512 * (blk + 1)], ADD, ADD,
                )
                dma = (
                    (nc.scalar.dma_start if h == 1 else nc.sync.dma_start)
                    if (tail or tail_mode["on"])
                    else nc.sync.dma_start
                )
                dma(
                    out=outT[128 * h : 128 * (h + 1), 512 * blk : 512 * (blk + 1)],
                    in_=osb[:],
                )

        return _Piece(
            lambda: aux.tile([128, 512], F32, tag="aux", name=f"o_{it}_{round_}_{h}_{blk}"),
            [mk_mm(r) for r in range(4)], fin,
        )

    # ---------------- per-pass post-processing ----------------
    def pp_stage1(pi, j, q0, w, w2_t, tail=False):
        """Evacuate w2 (+row-sum row) to SBUF bf16; start 1/rowsum chain.
        In the tail the reciprocal leads (it gates the final mul); mid-kernel
        the evac leads (it frees the w2 psum banks for the next pass)."""
        w2sb = w2sbp.tile([Dh, w], BF16, tag="w2sb", name=f"w2sb_{it}_{pi}")
        rs_row = rsp.tile([1, w], F32, tag="rsrow", bufs=2, name=f"rsrow_{it}_{pi}")
        rsr = rsp.tile([1, w], F32, tag="rsr", bufs=2, name=f"rsr_{it}_{pi}")
        if tail:
            # ACT is idle after the last exp: it fetches the row-sum row
            # while DVE evacuates the values, shortening the serial chain
            nc.scalar.copy(rs_row[:], w2_t[Dh : Dh + 1, :])
            nc.vector.reciprocal_approx_fast(out=rsr[:], in_=rs_row[:])
            nc.vector.tensor_copy(w2sb[:], w2_t[0:Dh, :])
        else:
            nc.vector.tensor_copy(w2sb[:], w2_t[0:Dh, :])
            nc.vector.tensor_copy(rs_row[:], w2_t[Dh : Dh + 1, :])
            nc.vector.reciprocal_approx_fast(out=rsr[:], in_=rs_row[:])
        rs_b = rsp.tile([Dh, w], F32, tag="rsb", bufs=2, name=f"rsb_{it}_{pi}")
        nc.gpsimd.partition_broadcast(rs_b[:], rsr[:])
        return w2sb, rs_b

    def ctx_dma(j, q0, w, ctxT):
        """Write normalized context into the collective input buffers."""
        if j < 2:
            c0 = q0 % SH
            nc.sync.dma_start(
                out=in_cc_h[q0 // SH][Dh * j : Dh * (j + 1), c0 : c0 + w],
                in_=ctxT[:],
            )
            return
        row = Dh * (j - 2)
        for lo, hi, cont in (
            (0, SH, in_cc2h0),
            (SH, SH + 512, in_cc2q[0]),
            (SH + 512, S, in_cc2q[1]),
        ):
            a, b = max(q0, lo), min(q0 + w, hi)
            if a < b:
                nc.sync.dma_start(
                    out=cont[row : row + Dh, a - lo : b - lo],
                    in_=ctxT[:, a - q0 : b - q0],
                )

    def z_pieces(pi, j, q0, w, w2sb, rs_b, after):
        ctxT = ctxp.tile([Dh, w], BF16, tag="ctxT", name=f"ctxT_{it}_{pi}")
        n = w // 512
        done = [0]

        def mk_mm(t):
            def mm(tl):
                nc.tensor.matmul(
                    tl[:], wvT_sb[:], w2sb[0:Dh, 512 * t : 512 * (t + 1)],
                    start=True, stop=True,
                )
            return mm

        def mk_fin(t):
            def fin(tl):
                nc.vector.tensor_mul(
                    ctxT[:, 512 * t : 512 * (t + 1)], tl[:],
                    rs_b[:, 512 * t : 512 * (t + 1)],
                )
                done[0] += 1
                if done[0] == n:
                    if _DEBUG and pi == 0:
                        dbc = persist.tile([Dh, SH], F32, tag="dbc")
                        nc.vector.tensor_copy(dbc[:], ctxT[:])
                        nc.sync.dma_start(out=io["dbg_ctx"][:, :], in_=dbc[:])
                    ctx_dma(j, q0, w, ctxT)
                    if after is not None:
                        after()
            return fin

        return [
            _Piece(
                lambda t=t: aux.tile([Dh, 512], F32, tag="aux", name=f"z_{it}_{pi}_{t}"),
                [mk_mm(t)], mk_fin(t),
            )
            for t in range(n)
        ]

    # ---------------- main pass loop ----------------
    # (head, q-start, width); the last q-half runs as two quarter passes so
    # its AllGather/out-projection chain overlaps the final attention work.
    Q = 512
    passes = (
        [(0, Q * i, Q) for i in range(4)]
        + [(1, Q * i, Q) for i in range(4)]
        + [(2, 0, Q), (2, Q, Q), (3, 0, Q), (3, Q, Q)]
        + [(2, 2 * Q, Q), (2, 3 * Q, Q), (3, 2 * Q, Q), (3, 3 * Q, Q)]
    )
    # AllGather trigger after the context of a given pass lands
    ag_after = {
        5: lambda: emit_ag0(0), 7: lambda: emit_ag0(1),
        11: emit_ag2h0, 14: lambda: emit_agq(0), 15: lambda: emit_agq(1),
    }
    u_after = {2: 1, 6: 2, 9: 3}   # pass index -> head whose u to prefetch

    fillers = _Fillers()
    state = {"pp": None, "z": None}

    def emit_pp(pi_now, tail=False):
        pj, pq0, pw, pw2, ppi = state["pp"]
        state["pp"] = None
        w2sb, rs_b = pp_stage1(ppi, pj, pq0, pw, pw2, tail=tail)
        if _DEBUG and ppi == 0:
            dbw = persist.tile([Dh, SH], F32, tag="dbw")
            nc.vector.tensor_copy(dbw[:], w2sb[:])
            nc.sync.dma_start(out=io["dbg_w2"][:, :], in_=dbw[:])
            nc.sync.dma_start(out=io["dbg_rs"][:, :], in_=rs_b[:])
        state["z"] = z_pieces(ppi, pj, pq0, pw, w2sb, rs_b, ag_after.get(ppi))

    def sched(pi, m):
        j, q0, w = passes[pi]
        if m == 2 and state["z"] is not None:
            for p in reversed(state["z"]):
                fillers.add(p, front=True)
            state["z"] = None
        if m == 2 and pi in u_after:
            fillers.finish_front()
        if m == 3 and pi in u_after:
            for t in reversed(range(4)):
                fillers.add(u_piece(u_after[pi], t), front=True)
        if pi == 0 and m == 0:
            for t in (2, 3):
                fillers.add(u_piece(0, t), front=True)
        if pi == 8 and m == 2:
            for h in range(2):
                for blk in (0, 1):
                    fillers.add(o_piece(0, h, blk))
        if pi == 10 and m == 2:
            for h in range(2):
                for blk in (2, 3):
                    fillers.add(o_piece(0, h, blk))
        if pi == 13 and m == 2:
            for h in range(2):
                fillers.add(o_piece(1, h, 0))
        if pi == 14 and m == 1:
            for h in range(2):
                fillers.add(o_piece(1, h, 1))

    # Each unit covers TWO key chunks for a 512-wide q-range: the scores
    # land side by side in one [128, 2, 512] psum tile so a single
    # [128,1024]-wide exp instruction serves both chunks (amortizing ACT's
    # per-instruction overhead).  Scores are emitted one unit ahead (two at
    # pass boundaries); w2 is double-buffered so boundaries need no
    # turnaround slack.
    NK2 = NK // 2
    units = [
        (pi, j, q0, w, m2)
        for pi, (j, q0, w) in enumerate(passes)
        for m2 in range(NK2)
    ]
    sc_tiles = {}
    w2_tiles = {}
    w2_thunks = {}

    def emit_sc(n):
        pi, j, q0, w, m2 = units[n]
        sc = scp.tile([128, 2, 512], F32, tag="sc", name=f"sc_{it}_{pi}_{m2}")
        for c in range(2):
            mm = 2 * m2 + c
            nc.tensor.matmul(
                sc[:, c, :],
                kT[:, j, 128 * mm : 128 * (mm + 1)],
                u_bf[j % 2][:, q0 : q0 + 512],
                start=True, stop=True,
            )
        sc_tiles[n] = sc

    emit_sc(0)
    for n, (pi, j, q0, w, m2) in enumerate(units):
        if n + 1 < len(units) and n + 1 not in sc_tiles:
            emit_sc(n + 1)
        if m2 >= NK2 - 2 and n + 2 < len(units):
            emit_sc(n + 2)
        if m2 == 0 and pi > 0:
            w2_thunks.pop((pi - 1, NK2 - 1))()  # previous pass's last w2
        if m2 == 0:
            w2_tiles[pi] = w2p.tile([Dh + 1, 512], F32, tag="w2", name=f"w2_{it}_{pi}")
        if m2 > 0:
            w2_thunks.pop((pi, m2 - 1))()
        if m2 == 0 and state["pp"] is not None:
            emit_pp(pi)
        sched(pi, m2)
        if 0 < m2 < NK2 - 1:
            fillers.pop_one()
        p_bf = ppool.tile([128, 2, 512], BF16, tag="p", name=f"p_{it}_{pi}_{m2}")
        nc.scalar.activation(p_bf[:], sc_tiles.pop(n)[:], EXP, scale=0.125)

        def mk_w2(m2_=m2, p_=p_bf, pi_=pi, j_=j):
            def go():
                for c in range(2):
                    mm = 2 * m2_ + c
                    nc.tensor.matmul(
                        w2_tiles[pi_][:, :],
                        vin[:, j_, mm, :], p_[:, c, :],
                        start=(m2_ == 0 and c == 0),
                        stop=(m2_ == NK2 - 1 and c == 1),
                    )
            return go

        w2_thunks[(pi, m2)] = mk_w2()
        if m2 == NK2 - 1:
            state["pp"] = (j, q0, w, w2_tiles[pi], pi)

    # ---------------- tail ----------------
    w2_thunks.pop((len(passes) - 1, NK2 - 1))()
    emit_pp(None, tail=True)                   # rsr/evac/bcast first on DVE
    zs = state["z"]
    state["z"] = None
    for p in zs:                               # z + ctx DMA + final AllGather
        while not p.step():
            pass
    fillers.drain()                            # leftover half-0 round-1 pieces
    # quarter-0 out projection keeps PE busy while the final chain flies
    tailq0 = [o_piece(1, h, 2, tail=True) for h in range(2)]
    for r in range(4):
        for p in tailq0:
            p.step()
    # PE keep-warm chain across the AllGather latency: short PE->DVE->PE
    # round-trips every ~1.5us so the tensor engine's p-state never drops
    # before the final out-projection matmuls.
    warm_src = None
    for i in range(3):
        wt = aux.tile([Dh, 512], F32, tag="aux", name=f"warm_{it}_{i}")
        nc.tensor.matmul(
            wt[:], wvT_sb[:],
            warm_src if warm_src is not None else u_bf[1][:, 0:512],
            start=True, stop=True,
        )
        if i < 2:
            ws = persist.tile([Dh, 512], BF16, tag=f"warm{i}", name=f"wsc_{it}_{i}")
            nc.vector.tensor_copy(ws[:], wt[:])
            warm_src = ws[:]
    # final out-projection quarter: interleave the two pieces so their
    # accumulating matmuls pipeline behind the per-core staged slices
    tailp = [o_piece(1, h, 3, tail=True) for h in range(2)]
    for r in range(4):
        for p in tailp:
            p.step()


def _build(repeats=1, collective=True):
    key = (repeats, collective)
    if key in _CACHE:
        return _CACHE[key]
    ndev = N_CORES if collective else 1
    nc = bacc.Bacc("TRN2", target_bir_lowering=False, debug=False, num_devices=ndev)
    io = _declare_io(nc)
    with tile.TileContext(nc) as tc:
        for it in range(repeats):
            with contextlib.ExitStack() as es:
                _body(nc, tc, es, io, it, collective=collective)
    nc.compile()
    _CACHE[key] = nc
    return nc


def kernel(k_in, q_in, v_in, Wq, Wk, Wv, Wo, bo, _repeats=1, _results_hook=None):
    bf = ml_dtypes.bfloat16
    q_in = np.asarray(q_in, np.float32)
    k_in = np.asarray(k_in, np.float32)
    v_in = np.asarray(v_in, np.float32)
    Wq = np.asarray(Wq, np.float32)
    Wk = np.asarray(Wk, np.float32)
    Wv = np.asarray(Wv, np.float32)
    Wo = np.asarray(Wo, np.float32)
    bo = np.asarray(bo, np.float32)

    nc = _build(_repeats)

    wqk = np.ascontiguousarray(np.stack([Wq, Wk], axis=1))           # [64,2,64]
    wvT = np.ascontiguousarray(Wv.T.astype(bf))

    in_maps = []
    for c in range(N_CORES):
        b, q4 = c // 4, c % 4
        sl = slice(256 * q4, 256 * (q4 + 1))
        q_s = q_in[b, :, sl]                                         # [S, 256]
        k_s = k_in[b, :, sl]
        v_s = v_in[b, :, sl]
        vin = np.ones((128, HPC, NK, Dh + 1), np.float32)
        vin[:, :, :, :Dh] = v_s.reshape(NK, 128, HPC, Dh).transpose(1, 2, 0, 3)
        in_maps.append(
            {
                "qT": np.ascontiguousarray(
                    q_s.T.reshape(HPC, Dh, S).transpose(1, 0, 2).astype(bf)
                ),
                "kT": np.ascontiguousarray(
                    k_s.T.reshape(HPC, Dh, S).transpose(1, 0, 2).astype(bf)
                ),
                "vin": np.ascontiguousarray(vin.astype(bf)),
                "wqk": wqk,
                "wvT": wvT,
                "woT": np.ascontiguousarray(
                    Wo[sl, :].T.reshape(8, 128, EOUT).transpose(1, 0, 2).astype(bf)
                ),
                "bo2": np.ascontiguousarray(bo[sl].reshape(2, 128).T),
            }
        )

    res = run_bass_kernel_spmd(nc, in_maps, core_ids=list(range(N_CORES)))
    if _results_hook is not None:
        _results_hook(res)

    out = np.empty((B, S, E), dtype=np.float32)
    for c in range(N_CORES):
        b, q4 = c // 4, c % 4
        out[b, :, 256 * q4 : 256 * (q4 + 1)] = res.results[c]["outT"].T
    return out
